# revision 45
# baseline (speedup 1.0000x reference)
"""Gaussian-kernel (Nadaraya-Watson) regression on 8 TRN2 NeuronCores.

Reference: out[q,d] = sum_n Y[n]*K / sum_n K, K = exp(-2*(proj[n,d]-xw[q,d])^2),
proj = train_X @ W.T [N,3], xw = x @ W.T [B,3], N=200000, B=256, H=0.5.

Algorithm (Fourier / fast-Gauss): periodize the 1-D kernel with period P and
truncate its cosine series at M terms:
    exp(-2*D^2) ~= sum_m a_m cos(w_m D),  w_m = 2*pi*m/P
    cos(w_m (p-c)) = cos(w_m p)cos(w_m c) + sin(w_m p)sin(w_m c)
so each core only computes trig MOMENTS of its N-shard:
    mom[w, (m,phi,d)] = sum_n {1,y_n} * {cos,sin}(w_m * proj[n,d])
and the host combines the 8 partial moments and evaluates the tiny [B,3]
query-side sum in f64. P=12, M=17 -> rel err ~1e-3 (fp16 pipeline).

Device pipeline per core (N-shard 25000 padded to 25088 = 196*128).
SC is m-major [128, 17, 196, 6]: each m-group SC[:,m] is one CONTIGUOUS
[128, 1176] block (DVE fp16 tensor_tensor needs long packed runs for its 2x
rate), while the PE rhs SC[:, :, c, :] reads 12-byte runs (vs 2-byte with a
rows-major layout, which streams ~4x slower):
  - host sends TH = [pi/2-|th| (3) | th (3)], THR = [pi/2-|th|]*2, th =
    w0*clip(p); ONE ACT Sin pass each: B6 = [cos1|sin1], B6R = [cos1|cos1]
  - DVE: scaled Chebyshev in fp16, u_m = 2*cos(m th) (+ sin partner):
      SC[:,1] = B6 + B6; U1R = B6R + B6R; SC[:,0] = memset [2,2,2,0,0,0]
      t = SC[:,m-1] . U1R ; SC[:,m] = t - SC[:,m-2]
  - PE: per chunk c: mom[2, cols] += Y2[:,2c:2c+2].T @ SC[:, m-range, c, :]
    (fp16, f32 PSUM accumulate; lhsT col0 = valid mask kills padding), in two
    m-panels so PE overlaps the tail of the recurrence.
Host divides all moments by 2 (u-scaling).
"""

import os
from contextlib import ExitStack

import numpy as np

import concourse.bass as bass
import concourse.tile as tile
from concourse import mybir
from concourse.bass_utils import run_bass_kernel_spmd

N_CORES = 8
B = 256
D = 3
N_TOTAL = 200000
N_SHARD = N_TOTAL // N_CORES  # 25000
CHUNK = 128
N_CHUNKS = (N_SHARD + CHUNK - 1) // CHUNK  # 196
N_PAD = N_CHUNKS * CHUNK  # 25088

P_PERIOD = 10.5
M_FREQ = 13
W0 = 2.0 * np.pi / P_PERIOD
ROWS = 6 * M_FREQ  # 78; row = m*6 + phi*3 + d
P_CLIP = P_PERIOD / 2 - 0.02  # |w0*p| < pi

# PE panel boundaries (m indices); panel i covers [PANELS[i], PANELS[i+1])
PANELS = [0] + [
    int(s) for s in os.environ.get("KNN_PANELS", "6").split(",") if s
] + [M_FREQ]

_nc_cache = {}
LAST_RESULTS = None


def _build_nc():
    f32 = mybir.dt.float32
    f16 = mybir.dt.float16
    nc = bass.Bass(trn_type="TRN2")

    TH_d = nc.dram_tensor("TH", [CHUNK, N_CHUNKS, 6], f16, kind="ExternalInput")
    Y2_d = nc.dram_tensor("Y2", [CHUNK, 2 * N_CHUNKS], f16, kind="ExternalInput")
    out_d = nc.dram_tensor("out", [2, 2 * ROWS], f32, kind="ExternalOutput")

    Alu = mybir.AluOpType
    Act = mybir.ActivationFunctionType
    HALF = N_CHUNKS // 2  # 98

    with ExitStack() as ctx:
        tc = ctx.enter_context(tile.TileContext(nc))
        const = ctx.enter_context(tc.tile_pool(name="const", bufs=1))
        tpool = ctx.enter_context(tc.tile_pool(name="tpool", bufs=2))
        spool = ctx.enter_context(tc.tile_pool(name="spool", bufs=2))
        mpool = ctx.enter_context(tc.tile_pool(name="mpool", bufs=1, space="PSUM"))

        # TH in thirds, one per DMA-capable queue
        edges = [round(i * N_CHUNKS / 3) for i in range(4)]
        T6 = list(zip(edges[:-1], edges[1:]))
        qs = [nc.scalar, nc.sync, nc.gpsimd]
        TH_t = const.tile([CHUNK, N_CHUNKS, 6], f16)
        for i, (c0, c1) in enumerate(T6):
            qs[i % 3].dma_start(out=TH_t[:, c0:c1, :], in_=TH_d[:, c0:c1, :])
        Y2_t = const.tile([CHUNK, 2 * N_CHUNKS], f16)
        nc.sync.dma_start(out=Y2_t[:], in_=Y2_d[:])

        SC_t = const.tile([CHUNK, M_FREQ, N_CHUNKS, 6], f16)
        B6_t = const.tile([CHUNK, N_CHUNKS, 6], f16)  # [cos1(3) | sin1(3)]
        U1R_t = const.tile([CHUNK, N_CHUNKS, 6], f16)  # [2c1(3) | 2c1(3)]

        # m0 group doubles as u_0 = [2,2,2,0,0,0] and yields count / sum(y)
        nc.gpsimd.memset(SC_t[:, 0, :, 0:3], 2.0)
        nc.gpsimd.memset(SC_t[:, 0, :, 3:6], 0.0)

        # base: one Sin pass per input piece (args within +-pi by construction)
        for c0, c1 in T6:
            nc.scalar.activation(B6_t[:, c0:c1, :], TH_t[:, c0:c1, :], Act.Sin)

        # U1R = [2c1 | 2c1]: two strided doublings of the cos half
        nc.vector.tensor_tensor(
            U1R_t[:, :, 0:3], B6_t[:, :, 0:3], B6_t[:, :, 0:3], Alu.add
        )
        nc.vector.tensor_tensor(
            U1R_t[:, :, 3:6], B6_t[:, :, 0:3], B6_t[:, :, 0:3], Alu.add
        )
        nc.vector.tensor_tensor(SC_t[:, 1], B6_t[:], B6_t[:], Alu.add)

        def cosr(m):
            return SC_t[:, m, :, 0:3]

        def sinr(m):
            return SC_t[:, m, :, 3:6]

        def emit_cheb(m):  # odd m: full Chebyshev pair step on DVE
            t = tpool.tile([CHUNK, N_CHUNKS, 6], f16)
            nc.vector.tensor_tensor(t[:], SC_t[:, m - 1], U1R_t[:], Alu.mult)
            nc.vector.tensor_tensor(SC_t[:, m], t[:], SC_t[:, m - 2], Alu.subtract)

        def emit_even_sin(m, eng=None):  # u_s_2k = u_s_k * u_c_k
            k = m // 2
            (eng or nc.vector).tensor_tensor(sinr(m), sinr(k), cosr(k), Alu.mult)

        def emit_even_cos(m):  # u_c_2k = 2 - u_s_k^2 on ACT
            k = m // 2
            sq = spool.tile([CHUNK, N_CHUNKS, 3], f16)
            nc.scalar.activation(sq[:], sinr(k), Act.Square)
            nc.scalar.activation(cosr(m), sq[:], Act.Copy, scale=-1.0, bias=2.0)

        # double-angle tree: even cos on ACT, even sin on DVE (2,4) and the
        # otherwise-idle GpSimd (6..12), odd pairs on DVE. Emission order per
        # engine minimizes in-order stalls.
        emit_even_cos(2)
        nc.vector.tensor_tensor(sinr(2), sinr(1), cosr(1), Alu.mult)
        emit_even_sin(4)  # s4 = s2*c2 (waits ACT c2)
        emit_even_cos(4)
        emit_cheb(3)

        def emit_even_pair(k0):  # evens {2k0, 2k0+2} from halves {k0, k0+1}
            sq = spool.tile([CHUNK, 2, N_CHUNKS, 3], f16)
            nc.scalar.activation(sq[:], SC_t[:, k0 : k0 + 2, :, 3:6], Act.Square)
            nc.scalar.activation(
                SC_t[:, 2 * k0 : 2 * k0 + 3 : 2, :, 0:3],
                sq[:],
                Act.Copy,
                scale=-1.0,
                bias=2.0,
            )
            nc.vector.tensor_tensor(
                SC_t[:, 2 * k0 : 2 * k0 + 3 : 2, :, 3:6],
                SC_t[:, k0 : k0 + 2, :, 3:6],
                SC_t[:, k0 : k0 + 2, :, 0:3],
                Alu.mult,
            )

        emit_even_pair(3)  # 6, 8 from 3, 4
        emit_cheb(5)
        emit_even_pair(5)  # 10, 12 from 5, 6
        emit_cheb(7)
        emit_cheb(9)
        emit_cheb(11)

        def emit_panel(momA, momB, m0, m1):
            for c in range(N_CHUNKS):
                mom = momA if c % 2 == 0 else momB
                nc.tensor.matmul(
                    mom[:, m0 * 6 : m1 * 6],
                    lhsT=Y2_t[:, 2 * c : 2 * c + 2],
                    rhs=SC_t[:, m0:m1, c, :],
                    start=(c < 2),
                    stop=(c >= N_CHUNKS - 2),
                )

        momA = mpool.tile([2, ROWS], f32)
        momB = mpool.tile([2, ROWS], f32)
        for p0, p1 in zip(PANELS[:-1], PANELS[1:]):
            emit_panel(momA, momB, p0, p1)

        o_t = const.tile([2, 2 * ROWS], f32)
        nc.vector.tensor_copy(o_t[:, 0:ROWS], momA[:])
        nc.vector.tensor_copy(o_t[:, ROWS : 2 * ROWS], momB[:])
        nc.scalar.dma_start(out=out_d[:], in_=o_t[:])

    _strip_self_waits(nc)
    _bulk_pe_updates(nc)
    _split_multi_waits(nc)
    return nc


def _bulk_pe_updates(nc):
    """Replace per-matmul semaphore increments with one bulk increment.

    Every mm2 matmul sem-incs the PE semaphore; the only waiters need the
    final count. The ~14ns/instr sem-send overhead is pure loss on 392
    back-to-back matmuls, so strip the updates and let the LAST matmul
    increment by the full amount.
    """
    import bass_rust

    insts = [i for bb in nc.main_func.blocks for i in bb.instructions]
    by_sem = {}
    for i in insts:
        si = getattr(i, "sync_info", None)
        if si is None:
            continue
        for u in si.on_update:
            by_sem.setdefault(u.id, []).append((i, u))
    for sem, ups in by_sem.items():
        if len(ups) < 32:
            continue
        if not all(
            type(i).__name__ == "InstMatmult" and u.update_mode == "sem-inc"
            for i, u in ups
        ):
            continue
        for i, _ in ups[:-1]:
            si = i.sync_info
            i.sync_info = bass_rust.SyncInfo(
                on_wait=list(si.on_wait),
                on_update=[u for u in si.on_update if u.id != sem],
            )
        # single remaining inc-by-1: every waiter just needs "the last MM
        # retired" (a superset of its original dependency)
        for i in insts:
            si = getattr(i, "sync_info", None)
            if si is None:
                continue
            changed = False
            for w in si.on_wait:
                if w.id == sem and (w.wait_value or 0) > 1:
                    w.wait_value = 1
                    changed = True
            if changed:
                i.sync_info = bass_rust.SyncInfo(
                    on_wait=list(si.on_wait), on_update=list(si.on_update)
                )


def _split_multi_waits(nc):
    """Walrus encodes at most one sync-wait per instruction on this target.

    Move all but the last wait of any multi-wait instruction onto preceding
    same-engine NoOps (in-order queues make sequential waiting equivalent to
    the ANDed wait set).
    """
    import bass_rust

    for bb_holder in nc.main_func.blocks:
        insts = list(bb_holder.instructions)
        out = []
        changed = False
        for i in insts:
            si = getattr(i, "sync_info", None)
            if (
                si is not None
                and len(si.on_wait) > 1
                and type(i).__name__ != "InstEventSemaphore"
            ):
                for w in si.on_wait[:-1]:
                    nop = mybir.InstNoOp(
                        name=nc.get_next_instruction_name(),
                        sync_info=bass_rust.SyncInfo(on_wait=[w], on_update=[]),
                        bass_nofuse=True,
                        engine=i.engine,
                    )
                    out.append(nop)
                i.sync_info = bass_rust.SyncInfo(
                    on_wait=[si.on_wait[-1]], on_update=list(si.on_update)
                )
                changed = True
            out.append(i)
        if changed:
            _replace_bb_instructions(bb_holder, out)


def _replace_bb_instructions(bb_holder, new_insts):
    bb = getattr(bb_holder, "bb", bb_holder)
    try:
        bb.instructions = new_insts
    except Exception:
        while len(bb.instructions):
            bb.instructions.pop()
        for x in new_insts:
            bb.add_instruction(x)


def _strip_self_waits(nc):
    """Drop semaphore waits that an in-order engine holds against itself.

    Tile emits WAW waits (e.g. temp-pool slot reuse) on the engine's own
    semaphore. In-order queues always satisfy these, but they push the
    per-instruction sync-wait count past what walrus codegen encodes.
    Only waits on semaphores updated exclusively by same-engine instructions
    are removed, and only for in-order engines (PE reorders LDWEIGHTS).
    """
    import bass_rust

    SAFE = (mybir.EngineType.Activation, mybir.EngineType.DVE, mybir.EngineType.Pool)
    insts = [i for bb in nc.main_func.blocks for i in bb.instructions]
    updaters = {}
    for i in insts:
        si = getattr(i, "sync_info", None)
        if si is None:
            continue
        for u in si.on_update:
            updaters.setdefault(u.id, set()).add(i.engine)
    for i in insts:
        if i.engine not in SAFE:
            continue
        si = getattr(i, "sync_info", None)
        if si is None or len(si.on_wait) <= 1:
            continue
        keep = [w for w in si.on_wait if updaters.get(w.id, {None}) != {i.engine}]
        if len(keep) != len(si.on_wait):
            i.sync_info = bass_rust.SyncInfo(
                on_wait=keep, on_update=list(si.on_update)
            )


def _get_nc():
    if "nc" not in _nc_cache:
        _nc_cache["nc"] = _build_nc()
    return _nc_cache["nc"]


def kernel(x, train_X, Y, W):
    global LAST_RESULTS
    x = np.ascontiguousarray(np.asarray(x, dtype=np.float32))
    train_X = np.ascontiguousarray(np.asarray(train_X, dtype=np.float32))
    Y = np.ascontiguousarray(np.asarray(Y, dtype=np.float32))
    W = np.ascontiguousarray(np.asarray(W, dtype=np.float32))

    xw = (x @ W.T).astype(np.float64)  # [B,3]
    th = (W0 * np.clip(train_X @ W.T, -P_CLIP, P_CLIP)).astype(np.float32)
    thc = (np.pi / 2 - np.abs(th)).astype(np.float32)

    in_maps = []
    for s in range(N_CORES):
        th6 = np.zeros((N_PAD, 6), dtype=np.float16)
        th6[:N_SHARD, 0:3] = thc[s * N_SHARD : (s + 1) * N_SHARD]
        th6[N_SHARD:, 0:3] = np.pi / 2
        th6[:N_SHARD, 3:6] = th[s * N_SHARD : (s + 1) * N_SHARD]
        TH = np.ascontiguousarray(th6.reshape(N_CHUNKS, CHUNK, 6).transpose(1, 0, 2))

        y2 = np.zeros((N_PAD, 2), dtype=np.float16)
        y2[:N_SHARD, 0] = 1.0
        y2[:N_SHARD, 1] = Y[s * N_SHARD : (s + 1) * N_SHARD].astype(np.float16)
        Y2 = np.ascontiguousarray(
            y2.reshape(N_CHUNKS, CHUNK, 2).transpose(1, 0, 2).reshape(CHUNK, -1)
        )
        in_maps.append({"TH": TH, "Y2": Y2})

    nc = _get_nc()
    res = run_bass_kernel_spmd(
        nc,
        in_maps,
        core_ids=list(range(N_CORES)),
        trace=bool(int(os.environ.get("KNN_TRACE", "0"))),
    )
    LAST_RESULTS = res

    raw = np.zeros((2, ROWS), dtype=np.float64)
    for r in res.results:
        o = r["out"].astype(np.float64)
        raw += o[:, 0:ROWS] + o[:, ROWS : 2 * ROWS]  # two PSUM banks
    raw *= 0.5  # u-scaling: device rows are 2*cos / 2*sin

    mom = raw.reshape(2, M_FREQ, 2, D)  # [w, m, phi, d]

    m = np.arange(M_FREQ)
    om = m * W0
    a = (2.0 - (m == 0)) / P_PERIOD * np.sqrt(np.pi / 2.0) * np.exp(-(om**2) / 8.0)
    qarg = om[None, None, :] * xw[:, :, None]  # [B,3,M]
    qc = np.cos(qarg)
    qs = np.sin(qarg)
    S0 = np.einsum("m,bdm->bd", a, qc * mom[0, :, 0, :].T[None]) + np.einsum(
        "m,bdm->bd", a, qs * mom[0, :, 1, :].T[None]
    )
    S1 = np.einsum("m,bdm->bd", a, qc * mom[1, :, 0, :].T[None]) + np.einsum(
        "m,bdm->bd", a, qs * mom[1, :, 1, :].T[None]
    )
    return (S1 / S0).astype(np.float32)


# revision 46
# speedup vs baseline: 1.0186x; 1.0186x over previous
"""Gaussian-kernel (Nadaraya-Watson) regression on 8 TRN2 NeuronCores.

Reference: out[q,d] = sum_n Y[n]*K / sum_n K, K = exp(-2*(proj[n,d]-xw[q,d])^2),
proj = train_X @ W.T [N,3], xw = x @ W.T [B,3], N=200000, B=256, H=0.5.

Algorithm (Fourier / fast-Gauss): periodize the 1-D kernel with period P and
truncate its cosine series at M terms:
    exp(-2*D^2) ~= sum_m a_m cos(w_m D),  w_m = 2*pi*m/P
    cos(w_m (p-c)) = cos(w_m p)cos(w_m c) + sin(w_m p)sin(w_m c)
so each core only computes trig MOMENTS of its N-shard:
    mom[w, (m,phi,d)] = sum_n {1,y_n} * {cos,sin}(w_m * proj[n,d])
and the host combines the 8 partial moments and evaluates the tiny [B,3]
query-side sum in f64. P=12, M=17 -> rel err ~1e-3 (fp16 pipeline).

Device pipeline per core (N-shard 25000 padded to 25088 = 196*128).
SC is m-major [128, 17, 196, 6]: each m-group SC[:,m] is one CONTIGUOUS
[128, 1176] block (DVE fp16 tensor_tensor needs long packed runs for its 2x
rate), while the PE rhs SC[:, :, c, :] reads 12-byte runs (vs 2-byte with a
rows-major layout, which streams ~4x slower):
  - host sends TH = [pi/2-|th| (3) | th (3)], THR = [pi/2-|th|]*2, th =
    w0*clip(p); ONE ACT Sin pass each: B6 = [cos1|sin1], B6R = [cos1|cos1]
  - DVE: scaled Chebyshev in fp16, u_m = 2*cos(m th) (+ sin partner):
      SC[:,1] = B6 + B6; U1R = B6R + B6R; SC[:,0] = memset [2,2,2,0,0,0]
      t = SC[:,m-1] . U1R ; SC[:,m] = t - SC[:,m-2]
  - PE: per chunk c: mom[2, cols] += Y2[:,2c:2c+2].T @ SC[:, m-range, c, :]
    (fp16, f32 PSUM accumulate; lhsT col0 = valid mask kills padding), in two
    m-panels so PE overlaps the tail of the recurrence.
Host divides all moments by 2 (u-scaling).
"""

import os
from contextlib import ExitStack

import numpy as np

import concourse.bass as bass
import concourse.tile as tile
from concourse import mybir
from concourse.bass_utils import run_bass_kernel_spmd

N_CORES = 8
B = 256
D = 3
N_TOTAL = 200000
N_SHARD = N_TOTAL // N_CORES  # 25000
CHUNK = 128
N_CHUNKS = (N_SHARD + CHUNK - 1) // CHUNK  # 196
N_PAD = N_CHUNKS * CHUNK  # 25088

P_PERIOD = 10.5
M_FREQ = 13
W0 = 2.0 * np.pi / P_PERIOD
ROWS = 6 * M_FREQ  # 78; row = m*6 + phi*3 + d
P_CLIP = P_PERIOD / 2 - 0.02  # |w0*p| < pi

# PE panel boundaries (m indices); panel i covers [PANELS[i], PANELS[i+1])
PANELS = [0] + [
    int(s) for s in os.environ.get("KNN_PANELS", "6").split(",") if s
] + [M_FREQ]

_nc_cache = {}
LAST_RESULTS = None


def _build_nc():
    f32 = mybir.dt.float32
    f16 = mybir.dt.float16
    nc = bass.Bass(trn_type="TRN2")

    TH_d = nc.dram_tensor("TH", [CHUNK, N_CHUNKS, 6], f16, kind="ExternalInput")
    Y2_d = nc.dram_tensor("Y2", [CHUNK, 2 * N_CHUNKS], f16, kind="ExternalInput")
    out_d = nc.dram_tensor("out", [2, 2 * ROWS], f32, kind="ExternalOutput")

    Alu = mybir.AluOpType
    Act = mybir.ActivationFunctionType
    HALF = N_CHUNKS // 2  # 98

    with ExitStack() as ctx:
        tc = ctx.enter_context(tile.TileContext(nc))
        const = ctx.enter_context(tc.tile_pool(name="const", bufs=1))
        tpool = ctx.enter_context(tc.tile_pool(name="tpool", bufs=2))
        spool = ctx.enter_context(tc.tile_pool(name="spool", bufs=2))
        mpool = ctx.enter_context(tc.tile_pool(name="mpool", bufs=1, space="PSUM"))

        # TH in thirds, one per DMA-capable queue
        edges = [round(i * N_CHUNKS / 3) for i in range(4)]
        T6 = list(zip(edges[:-1], edges[1:]))
        qs = [nc.scalar, nc.sync, nc.gpsimd]
        TH_t = const.tile([CHUNK, N_CHUNKS, 6], f16)
        for i, (c0, c1) in enumerate(T6):
            qs[i % 3].dma_start(out=TH_t[:, c0:c1, :], in_=TH_d[:, c0:c1, :])
        Y2_t = const.tile([CHUNK, 2 * N_CHUNKS], f16)
        nc.sync.dma_start(out=Y2_t[:], in_=Y2_d[:])

        SC_t = const.tile([CHUNK, M_FREQ, N_CHUNKS, 6], f16)
        B6_t = const.tile([CHUNK, N_CHUNKS, 6], f16)  # [cos1(3) | sin1(3)]
        U1R_t = const.tile([CHUNK, N_CHUNKS, 6], f16)  # [2c1(3) | 2c1(3)]

        # m0 group doubles as u_0 = [2,2,2,0,0,0] and yields count / sum(y)
        nc.gpsimd.memset(SC_t[:, 0, :, 0:3], 2.0)
        nc.gpsimd.memset(SC_t[:, 0, :, 3:6], 0.0)

        # base: one Sin pass per input piece (args within +-pi by construction)
        for c0, c1 in T6:
            nc.scalar.activation(B6_t[:, c0:c1, :], TH_t[:, c0:c1, :], Act.Sin)

        # U1R = [2c1 | 2c1]: two strided doublings of the cos half
        nc.vector.tensor_tensor(
            U1R_t[:, :, 0:3], B6_t[:, :, 0:3], B6_t[:, :, 0:3], Alu.add
        )
        nc.vector.tensor_tensor(
            U1R_t[:, :, 3:6], B6_t[:, :, 0:3], B6_t[:, :, 0:3], Alu.add
        )
        nc.vector.tensor_tensor(SC_t[:, 1], B6_t[:], B6_t[:], Alu.add)

        def cosr(m):
            return SC_t[:, m, :, 0:3]

        def sinr(m):
            return SC_t[:, m, :, 3:6]

        def emit_cheb(m):  # odd m: full Chebyshev pair step on DVE
            t = tpool.tile([CHUNK, N_CHUNKS, 6], f16)
            nc.vector.tensor_tensor(t[:], SC_t[:, m - 1], U1R_t[:], Alu.mult)
            nc.vector.tensor_tensor(SC_t[:, m], t[:], SC_t[:, m - 2], Alu.subtract)

        def emit_even_sin(m, eng=None):  # u_s_2k = u_s_k * u_c_k
            k = m // 2
            (eng or nc.vector).tensor_tensor(sinr(m), sinr(k), cosr(k), Alu.mult)

        def emit_even_cos(m):  # u_c_2k = 2 - u_s_k^2 on ACT
            k = m // 2
            sq = spool.tile([CHUNK, N_CHUNKS, 3], f16)
            nc.scalar.activation(sq[:], sinr(k), Act.Square)
            nc.scalar.activation(cosr(m), sq[:], Act.Copy, scale=-1.0, bias=2.0)

        # double-angle tree: even cos on ACT, even sin on DVE (2,4) and the
        # otherwise-idle GpSimd (6..12), odd pairs on DVE. Emission order per
        # engine minimizes in-order stalls.
        emit_even_cos(2)
        nc.vector.tensor_tensor(sinr(2), sinr(1), cosr(1), Alu.mult)
        emit_even_sin(4)  # s4 = s2*c2 (waits ACT c2)
        emit_even_cos(4)
        emit_cheb(3)
        emit_even_cos(6)  # from s3
        emit_even_sin(6)
        emit_even_sin(8)
        emit_even_cos(8)  # from s4
        emit_cheb(5)
        emit_even_cos(10)  # from s5
        emit_even_sin(10)
        emit_even_sin(12)
        emit_even_cos(12)  # from s6
        emit_cheb(7)
        emit_cheb(9)
        emit_cheb(11)

        def emit_panel(momA, momB, m0, m1):
            for c in range(N_CHUNKS):
                mom = momA if c % 2 == 0 else momB
                nc.tensor.matmul(
                    mom[:, m0 * 6 : m1 * 6],
                    lhsT=Y2_t[:, 2 * c : 2 * c + 2],
                    rhs=SC_t[:, m0:m1, c, :],
                    start=(c < 2),
                    stop=(c >= N_CHUNKS - 2),
                )

        momA = mpool.tile([2, ROWS], f32)
        momB = mpool.tile([2, ROWS], f32)
        for p0, p1 in zip(PANELS[:-1], PANELS[1:]):
            emit_panel(momA, momB, p0, p1)

        o_t = const.tile([2, 2 * ROWS], f32)
        nc.vector.tensor_copy(o_t[:, 0:ROWS], momA[:])
        nc.vector.tensor_copy(o_t[:, ROWS : 2 * ROWS], momB[:])
        nc.scalar.dma_start(out=out_d[:], in_=o_t[:])

    _strip_self_waits(nc)
    _bulk_pe_updates(nc)
    _split_multi_waits(nc)
    return nc


def _bulk_pe_updates(nc):
    """Replace per-matmul semaphore increments with one bulk increment.

    Every mm2 matmul sem-incs the PE semaphore; the only waiters need the
    final count. The ~14ns/instr sem-send overhead is pure loss on 392
    back-to-back matmuls, so strip the updates and let the LAST matmul
    increment by the full amount.
    """
    import bass_rust

    insts = [i for bb in nc.main_func.blocks for i in bb.instructions]
    by_sem = {}
    for i in insts:
        si = getattr(i, "sync_info", None)
        if si is None:
            continue
        for u in si.on_update:
            by_sem.setdefault(u.id, []).append((i, u))
    for sem, ups in by_sem.items():
        if len(ups) < 32:
            continue
        if not all(
            type(i).__name__ == "InstMatmult" and u.update_mode == "sem-inc"
            for i, u in ups
        ):
            continue
        for i, _ in ups[:-1]:
            si = i.sync_info
            i.sync_info = bass_rust.SyncInfo(
                on_wait=list(si.on_wait),
                on_update=[u for u in si.on_update if u.id != sem],
            )
        # single remaining inc-by-1: every waiter just needs "the last MM
        # retired" (a superset of its original dependency)
        for i in insts:
            si = getattr(i, "sync_info", None)
            if si is None:
                continue
            changed = False
            for w in si.on_wait:
                if w.id == sem and (w.wait_value or 0) > 1:
                    w.wait_value = 1
                    changed = True
            if changed:
                i.sync_info = bass_rust.SyncInfo(
                    on_wait=list(si.on_wait), on_update=list(si.on_update)
                )


def _split_multi_waits(nc):
    """Walrus encodes at most one sync-wait per instruction on this target.

    Move all but the last wait of any multi-wait instruction onto preceding
    same-engine NoOps (in-order queues make sequential waiting equivalent to
    the ANDed wait set).
    """
    import bass_rust

    for bb_holder in nc.main_func.blocks:
        insts = list(bb_holder.instructions)
        out = []
        changed = False
        for i in insts:
            si = getattr(i, "sync_info", None)
            if (
                si is not None
                and len(si.on_wait) > 1
                and type(i).__name__ != "InstEventSemaphore"
            ):
                for w in si.on_wait[:-1]:
                    nop = mybir.InstNoOp(
                        name=nc.get_next_instruction_name(),
                        sync_info=bass_rust.SyncInfo(on_wait=[w], on_update=[]),
                        bass_nofuse=True,
                        engine=i.engine,
                    )
                    out.append(nop)
                i.sync_info = bass_rust.SyncInfo(
                    on_wait=[si.on_wait[-1]], on_update=list(si.on_update)
                )
                changed = True
            out.append(i)
        if changed:
            _replace_bb_instructions(bb_holder, out)


def _replace_bb_instructions(bb_holder, new_insts):
    bb = getattr(bb_holder, "bb", bb_holder)
    try:
        bb.instructions = new_insts
    except Exception:
        while len(bb.instructions):
            bb.instructions.pop()
        for x in new_insts:
            bb.add_instruction(x)


def _strip_self_waits(nc):
    """Drop semaphore waits that an in-order engine holds against itself.

    Tile emits WAW waits (e.g. temp-pool slot reuse) on the engine's own
    semaphore. In-order queues always satisfy these, but they push the
    per-instruction sync-wait count past what walrus codegen encodes.
    Only waits on semaphores updated exclusively by same-engine instructions
    are removed, and only for in-order engines (PE reorders LDWEIGHTS).
    """
    import bass_rust

    SAFE = (mybir.EngineType.Activation, mybir.EngineType.DVE, mybir.EngineType.Pool)
    insts = [i for bb in nc.main_func.blocks for i in bb.instructions]
    updaters = {}
    for i in insts:
        si = getattr(i, "sync_info", None)
        if si is None:
            continue
        for u in si.on_update:
            updaters.setdefault(u.id, set()).add(i.engine)
    for i in insts:
        if i.engine not in SAFE:
            continue
        si = getattr(i, "sync_info", None)
        if si is None or len(si.on_wait) <= 1:
            continue
        keep = [w for w in si.on_wait if updaters.get(w.id, {None}) != {i.engine}]
        if len(keep) != len(si.on_wait):
            i.sync_info = bass_rust.SyncInfo(
                on_wait=keep, on_update=list(si.on_update)
            )


def _get_nc():
    if "nc" not in _nc_cache:
        _nc_cache["nc"] = _build_nc()
    return _nc_cache["nc"]


def kernel(x, train_X, Y, W):
    global LAST_RESULTS
    x = np.ascontiguousarray(np.asarray(x, dtype=np.float32))
    train_X = np.ascontiguousarray(np.asarray(train_X, dtype=np.float32))
    Y = np.ascontiguousarray(np.asarray(Y, dtype=np.float32))
    W = np.ascontiguousarray(np.asarray(W, dtype=np.float32))

    xw = (x @ W.T).astype(np.float64)  # [B,3]
    th = (W0 * np.clip(train_X @ W.T, -P_CLIP, P_CLIP)).astype(np.float32)
    thc = (np.pi / 2 - np.abs(th)).astype(np.float32)

    in_maps = []
    for s in range(N_CORES):
        th6 = np.zeros((N_PAD, 6), dtype=np.float16)
        th6[:N_SHARD, 0:3] = thc[s * N_SHARD : (s + 1) * N_SHARD]
        th6[N_SHARD:, 0:3] = np.pi / 2
        th6[:N_SHARD, 3:6] = th[s * N_SHARD : (s + 1) * N_SHARD]
        TH = np.ascontiguousarray(th6.reshape(N_CHUNKS, CHUNK, 6).transpose(1, 0, 2))

        y2 = np.zeros((N_PAD, 2), dtype=np.float16)
        y2[:N_SHARD, 0] = 1.0
        y2[:N_SHARD, 1] = Y[s * N_SHARD : (s + 1) * N_SHARD].astype(np.float16)
        Y2 = np.ascontiguousarray(
            y2.reshape(N_CHUNKS, CHUNK, 2).transpose(1, 0, 2).reshape(CHUNK, -1)
        )
        in_maps.append({"TH": TH, "Y2": Y2})

    nc = _get_nc()
    res = run_bass_kernel_spmd(
        nc,
        in_maps,
        core_ids=list(range(N_CORES)),
        trace=bool(int(os.environ.get("KNN_TRACE", "0"))),
    )
    LAST_RESULTS = res

    raw = np.zeros((2, ROWS), dtype=np.float64)
    for r in res.results:
        o = r["out"].astype(np.float64)
        raw += o[:, 0:ROWS] + o[:, ROWS : 2 * ROWS]  # two PSUM banks
    raw *= 0.5  # u-scaling: device rows are 2*cos / 2*sin

    mom = raw.reshape(2, M_FREQ, 2, D)  # [w, m, phi, d]

    m = np.arange(M_FREQ)
    om = m * W0
    a = (2.0 - (m == 0)) / P_PERIOD * np.sqrt(np.pi / 2.0) * np.exp(-(om**2) / 8.0)
    qarg = om[None, None, :] * xw[:, :, None]  # [B,3,M]
    qc = np.cos(qarg)
    qs = np.sin(qarg)
    S0 = np.einsum("m,bdm->bd", a, qc * mom[0, :, 0, :].T[None]) + np.einsum(
        "m,bdm->bd", a, qs * mom[0, :, 1, :].T[None]
    )
    S1 = np.einsum("m,bdm->bd", a, qc * mom[1, :, 0, :].T[None]) + np.einsum(
        "m,bdm->bd", a, qs * mom[1, :, 1, :].T[None]
    )
    return (S1 / S0).astype(np.float32)


# revision 47
# speedup vs baseline: 1.0389x; 1.0199x over previous
"""Gaussian-kernel (Nadaraya-Watson) regression on 8 TRN2 NeuronCores.

Reference: out[q,d] = sum_n Y[n]*K / sum_n K, K = exp(-2*(proj[n,d]-xw[q,d])^2),
proj = train_X @ W.T [N,3], xw = x @ W.T [B,3], N=200000, B=256, H=0.5.

Algorithm (Fourier / fast-Gauss): periodize the 1-D kernel with period P and
truncate its cosine series at M terms:
    exp(-2*D^2) ~= sum_m a_m cos(w_m D),  w_m = 2*pi*m/P
    cos(w_m (p-c)) = cos(w_m p)cos(w_m c) + sin(w_m p)sin(w_m c)
so each core only computes trig MOMENTS of its N-shard:
    mom[w, (m,phi,d)] = sum_n {1,y_n} * {cos,sin}(w_m * proj[n,d])
and the host combines the 8 partial moments and evaluates the tiny [B,3]
query-side sum in f64. P=12, M=17 -> rel err ~1e-3 (fp16 pipeline).

Device pipeline per core (N-shard 25000 padded to 25088 = 196*128).
SC is m-major [128, 17, 196, 6]: each m-group SC[:,m] is one CONTIGUOUS
[128, 1176] block (DVE fp16 tensor_tensor needs long packed runs for its 2x
rate), while the PE rhs SC[:, :, c, :] reads 12-byte runs (vs 2-byte with a
rows-major layout, which streams ~4x slower):
  - host sends TH = [pi/2-|th| (3) | th (3)], THR = [pi/2-|th|]*2, th =
    w0*clip(p); ONE ACT Sin pass each: B6 = [cos1|sin1], B6R = [cos1|cos1]
  - DVE: scaled Chebyshev in fp16, u_m = 2*cos(m th) (+ sin partner):
      SC[:,1] = B6 + B6; U1R = B6R + B6R; SC[:,0] = memset [2,2,2,0,0,0]
      t = SC[:,m-1] . U1R ; SC[:,m] = t - SC[:,m-2]
  - PE: per chunk c: mom[2, cols] += Y2[:,2c:2c+2].T @ SC[:, m-range, c, :]
    (fp16, f32 PSUM accumulate; lhsT col0 = valid mask kills padding), in two
    m-panels so PE overlaps the tail of the recurrence.
Host divides all moments by 2 (u-scaling).
"""

import os
from contextlib import ExitStack

import numpy as np

import concourse.bass as bass
import concourse.tile as tile
from concourse import mybir
from concourse.bass_utils import run_bass_kernel_spmd

N_CORES = 8
B = 256
D = 3
N_TOTAL = 200000
N_SHARD = N_TOTAL // N_CORES  # 25000
CHUNK = 128
N_CHUNKS = (N_SHARD + CHUNK - 1) // CHUNK  # 196
N_PAD = N_CHUNKS * CHUNK  # 25088

P_PERIOD = 10.5
M_FREQ = 13
W0 = 2.0 * np.pi / P_PERIOD
ROWS = 6 * M_FREQ  # 78; row = m*6 + phi*3 + d
P_CLIP = P_PERIOD / 2 - 0.02  # |w0*p| < pi

# PE panel boundaries (m indices); panel i covers [PANELS[i], PANELS[i+1])
PANELS = [0] + [
    int(s) for s in os.environ.get("KNN_PANELS", "6").split(",") if s
] + [M_FREQ]

_nc_cache = {}
LAST_RESULTS = None


def _build_nc():
    f32 = mybir.dt.float32
    f16 = mybir.dt.float16
    nc = bass.Bass(trn_type="TRN2")

    TH_d = nc.dram_tensor("TH", [CHUNK, N_CHUNKS, 6], f16, kind="ExternalInput")
    Y2_d = nc.dram_tensor("Y2", [CHUNK, 2 * N_CHUNKS], f16, kind="ExternalInput")
    out_d = nc.dram_tensor("out", [2, 2 * ROWS], f32, kind="ExternalOutput")

    Alu = mybir.AluOpType
    Act = mybir.ActivationFunctionType
    HALF = N_CHUNKS // 2  # 98

    with ExitStack() as ctx:
        tc = ctx.enter_context(tile.TileContext(nc))
        const = ctx.enter_context(tc.tile_pool(name="const", bufs=1))
        tpool = ctx.enter_context(tc.tile_pool(name="tpool", bufs=2))
        spool = ctx.enter_context(tc.tile_pool(name="spool", bufs=2))
        mpool = ctx.enter_context(tc.tile_pool(name="mpool", bufs=1, space="PSUM"))

        # TH in thirds, one per DMA-capable queue
        edges = [round(i * N_CHUNKS / 3) for i in range(4)]
        T6 = list(zip(edges[:-1], edges[1:]))
        qs = [nc.scalar, nc.sync, nc.gpsimd]
        TH_t = const.tile([CHUNK, N_CHUNKS, 6], f16)
        for i, (c0, c1) in enumerate(T6):
            qs[i % 3].dma_start(out=TH_t[:, c0:c1, :], in_=TH_d[:, c0:c1, :])
        Y2_t = const.tile([CHUNK, 2 * N_CHUNKS], f16)
        nc.sync.dma_start(out=Y2_t[:], in_=Y2_d[:])

        SC_t = const.tile([CHUNK, M_FREQ, N_CHUNKS, 6], f16)
        B6_t = const.tile([CHUNK, N_CHUNKS, 6], f16)  # [cos1(3) | sin1(3)]
        U1R_t = const.tile([CHUNK, N_CHUNKS, 6], f16)  # [2c1(3) | 2c1(3)]

        # m0 group doubles as u_0 = [2,2,2,0,0,0] and yields count / sum(y)
        nc.gpsimd.memset(SC_t[:, 0, :, 0:3], 2.0)
        nc.gpsimd.memset(SC_t[:, 0, :, 3:6], 0.0)

        # base: one Sin pass per input piece (args within +-pi by construction)
        for c0, c1 in T6:
            nc.scalar.activation(B6_t[:, c0:c1, :], TH_t[:, c0:c1, :], Act.Sin)

        # U1R = [2c1 | 2c1]: built on idle GpSimd, in parallel with DVE's u1
        nc.gpsimd.tensor_tensor(
            U1R_t[:, :, 0:3], B6_t[:, :, 0:3], B6_t[:, :, 0:3], Alu.add
        )
        nc.gpsimd.tensor_tensor(
            U1R_t[:, :, 3:6], B6_t[:, :, 0:3], B6_t[:, :, 0:3], Alu.add
        )
        nc.vector.tensor_tensor(SC_t[:, 1], B6_t[:], B6_t[:], Alu.add)

        def cosr(m):
            return SC_t[:, m, :, 0:3]

        def sinr(m):
            return SC_t[:, m, :, 3:6]

        def emit_cheb(m):  # odd m: full Chebyshev pair step on DVE
            t = tpool.tile([CHUNK, N_CHUNKS, 6], f16)
            nc.vector.tensor_tensor(t[:], SC_t[:, m - 1], U1R_t[:], Alu.mult)
            nc.vector.tensor_tensor(SC_t[:, m], t[:], SC_t[:, m - 2], Alu.subtract)

        def emit_even_sin(m, eng=None):  # u_s_2k = u_s_k * u_c_k
            k = m // 2
            (eng or nc.vector).tensor_tensor(sinr(m), sinr(k), cosr(k), Alu.mult)

        def emit_even_cos(m):  # u_c_2k = 2 - u_s_k^2 on ACT
            k = m // 2
            sq = spool.tile([CHUNK, N_CHUNKS, 3], f16)
            nc.scalar.activation(sq[:], sinr(k), Act.Square)
            nc.scalar.activation(cosr(m), sq[:], Act.Copy, scale=-1.0, bias=2.0)

        # double-angle tree: even cos on ACT, even sin on DVE (2,4) and the
        # otherwise-idle GpSimd (6..12), odd pairs on DVE. Emission order per
        # engine minimizes in-order stalls.
        emit_even_cos(2)
        nc.vector.tensor_tensor(sinr(2), sinr(1), cosr(1), Alu.mult)
        emit_even_sin(4)  # s4 = s2*c2 (waits ACT c2)
        emit_even_cos(4)
        emit_cheb(3)
        emit_even_cos(6)  # from s3
        emit_even_sin(6)
        emit_even_sin(8)
        emit_even_cos(8)  # from s4
        emit_cheb(5)
        emit_even_cos(10)  # from s5
        emit_even_sin(10)
        emit_even_sin(12)
        emit_even_cos(12)  # from s6
        emit_cheb(7)
        emit_cheb(9)
        emit_cheb(11)

        def emit_panel(momA, momB, m0, m1):
            for c in range(N_CHUNKS):
                mom = momA if c % 2 == 0 else momB
                nc.tensor.matmul(
                    mom[:, m0 * 6 : m1 * 6],
                    lhsT=Y2_t[:, 2 * c : 2 * c + 2],
                    rhs=SC_t[:, m0:m1, c, :],
                    start=(c < 2),
                    stop=(c >= N_CHUNKS - 2),
                )

        momA = mpool.tile([2, ROWS], f32)
        momB = mpool.tile([2, ROWS], f32)
        for p0, p1 in zip(PANELS[:-1], PANELS[1:]):
            emit_panel(momA, momB, p0, p1)

        o_t = const.tile([2, 2 * ROWS], f32)
        nc.vector.tensor_copy(o_t[:, 0:ROWS], momA[:])
        nc.vector.tensor_copy(o_t[:, ROWS : 2 * ROWS], momB[:])
        nc.scalar.dma_start(out=out_d[:], in_=o_t[:])

    _strip_self_waits(nc)
    _bulk_pe_updates(nc)
    _split_multi_waits(nc)
    return nc


def _bulk_pe_updates(nc):
    """Replace per-matmul semaphore increments with one bulk increment.

    Every mm2 matmul sem-incs the PE semaphore; the only waiters need the
    final count. The ~14ns/instr sem-send overhead is pure loss on 392
    back-to-back matmuls, so strip the updates and let the LAST matmul
    increment by the full amount.
    """
    import bass_rust

    insts = [i for bb in nc.main_func.blocks for i in bb.instructions]
    by_sem = {}
    for i in insts:
        si = getattr(i, "sync_info", None)
        if si is None:
            continue
        for u in si.on_update:
            by_sem.setdefault(u.id, []).append((i, u))
    for sem, ups in by_sem.items():
        if len(ups) < 32:
            continue
        if not all(
            type(i).__name__ == "InstMatmult" and u.update_mode == "sem-inc"
            for i, u in ups
        ):
            continue
        for i, _ in ups[:-1]:
            si = i.sync_info
            i.sync_info = bass_rust.SyncInfo(
                on_wait=list(si.on_wait),
                on_update=[u for u in si.on_update if u.id != sem],
            )
        # single remaining inc-by-1: every waiter just needs "the last MM
        # retired" (a superset of its original dependency)
        for i in insts:
            si = getattr(i, "sync_info", None)
            if si is None:
                continue
            changed = False
            for w in si.on_wait:
                if w.id == sem and (w.wait_value or 0) > 1:
                    w.wait_value = 1
                    changed = True
            if changed:
                i.sync_info = bass_rust.SyncInfo(
                    on_wait=list(si.on_wait), on_update=list(si.on_update)
                )


def _split_multi_waits(nc):
    """Walrus encodes at most one sync-wait per instruction on this target.

    Move all but the last wait of any multi-wait instruction onto preceding
    same-engine NoOps (in-order queues make sequential waiting equivalent to
    the ANDed wait set).
    """
    import bass_rust

    for bb_holder in nc.main_func.blocks:
        insts = list(bb_holder.instructions)
        out = []
        changed = False
        for i in insts:
            si = getattr(i, "sync_info", None)
            if (
                si is not None
                and len(si.on_wait) > 1
                and type(i).__name__ != "InstEventSemaphore"
            ):
                for w in si.on_wait[:-1]:
                    nop = mybir.InstNoOp(
                        name=nc.get_next_instruction_name(),
                        sync_info=bass_rust.SyncInfo(on_wait=[w], on_update=[]),
                        bass_nofuse=True,
                        engine=i.engine,
                    )
                    out.append(nop)
                i.sync_info = bass_rust.SyncInfo(
                    on_wait=[si.on_wait[-1]], on_update=list(si.on_update)
                )
                changed = True
            out.append(i)
        if changed:
            _replace_bb_instructions(bb_holder, out)


def _replace_bb_instructions(bb_holder, new_insts):
    bb = getattr(bb_holder, "bb", bb_holder)
    try:
        bb.instructions = new_insts
    except Exception:
        while len(bb.instructions):
            bb.instructions.pop()
        for x in new_insts:
            bb.add_instruction(x)


def _strip_self_waits(nc):
    """Drop semaphore waits that an in-order engine holds against itself.

    Tile emits WAW waits (e.g. temp-pool slot reuse) on the engine's own
    semaphore. In-order queues always satisfy these, but they push the
    per-instruction sync-wait count past what walrus codegen encodes.
    Only waits on semaphores updated exclusively by same-engine instructions
    are removed, and only for in-order engines (PE reorders LDWEIGHTS).
    """
    import bass_rust

    SAFE = (mybir.EngineType.Activation, mybir.EngineType.DVE, mybir.EngineType.Pool)
    insts = [i for bb in nc.main_func.blocks for i in bb.instructions]
    updaters = {}
    for i in insts:
        si = getattr(i, "sync_info", None)
        if si is None:
            continue
        for u in si.on_update:
            updaters.setdefault(u.id, set()).add(i.engine)
    for i in insts:
        if i.engine not in SAFE:
            continue
        si = getattr(i, "sync_info", None)
        if si is None or len(si.on_wait) <= 1:
            continue
        keep = [w for w in si.on_wait if updaters.get(w.id, {None}) != {i.engine}]
        if len(keep) != len(si.on_wait):
            i.sync_info = bass_rust.SyncInfo(
                on_wait=keep, on_update=list(si.on_update)
            )


def _get_nc():
    if "nc" not in _nc_cache:
        _nc_cache["nc"] = _build_nc()
    return _nc_cache["nc"]


def kernel(x, train_X, Y, W):
    global LAST_RESULTS
    x = np.ascontiguousarray(np.asarray(x, dtype=np.float32))
    train_X = np.ascontiguousarray(np.asarray(train_X, dtype=np.float32))
    Y = np.ascontiguousarray(np.asarray(Y, dtype=np.float32))
    W = np.ascontiguousarray(np.asarray(W, dtype=np.float32))

    xw = (x @ W.T).astype(np.float64)  # [B,3]
    th = (W0 * np.clip(train_X @ W.T, -P_CLIP, P_CLIP)).astype(np.float32)
    thc = (np.pi / 2 - np.abs(th)).astype(np.float32)

    in_maps = []
    for s in range(N_CORES):
        th6 = np.zeros((N_PAD, 6), dtype=np.float16)
        th6[:N_SHARD, 0:3] = thc[s * N_SHARD : (s + 1) * N_SHARD]
        th6[N_SHARD:, 0:3] = np.pi / 2
        th6[:N_SHARD, 3:6] = th[s * N_SHARD : (s + 1) * N_SHARD]
        TH = np.ascontiguousarray(th6.reshape(N_CHUNKS, CHUNK, 6).transpose(1, 0, 2))

        y2 = np.zeros((N_PAD, 2), dtype=np.float16)
        y2[:N_SHARD, 0] = 1.0
        y2[:N_SHARD, 1] = Y[s * N_SHARD : (s + 1) * N_SHARD].astype(np.float16)
        Y2 = np.ascontiguousarray(
            y2.reshape(N_CHUNKS, CHUNK, 2).transpose(1, 0, 2).reshape(CHUNK, -1)
        )
        in_maps.append({"TH": TH, "Y2": Y2})

    nc = _get_nc()
    res = run_bass_kernel_spmd(
        nc,
        in_maps,
        core_ids=list(range(N_CORES)),
        trace=bool(int(os.environ.get("KNN_TRACE", "0"))),
    )
    LAST_RESULTS = res

    raw = np.zeros((2, ROWS), dtype=np.float64)
    for r in res.results:
        o = r["out"].astype(np.float64)
        raw += o[:, 0:ROWS] + o[:, ROWS : 2 * ROWS]  # two PSUM banks
    raw *= 0.5  # u-scaling: device rows are 2*cos / 2*sin

    mom = raw.reshape(2, M_FREQ, 2, D)  # [w, m, phi, d]

    m = np.arange(M_FREQ)
    om = m * W0
    a = (2.0 - (m == 0)) / P_PERIOD * np.sqrt(np.pi / 2.0) * np.exp(-(om**2) / 8.0)
    qarg = om[None, None, :] * xw[:, :, None]  # [B,3,M]
    qc = np.cos(qarg)
    qs = np.sin(qarg)
    S0 = np.einsum("m,bdm->bd", a, qc * mom[0, :, 0, :].T[None]) + np.einsum(
        "m,bdm->bd", a, qs * mom[0, :, 1, :].T[None]
    )
    S1 = np.einsum("m,bdm->bd", a, qc * mom[1, :, 0, :].T[None]) + np.einsum(
        "m,bdm->bd", a, qs * mom[1, :, 1, :].T[None]
    )
    return (S1 / S0).astype(np.float32)
